# revision 51
# baseline (speedup 1.0000x reference)
"""DispNetC correlation volume on 8 NeuronCores (Trainium2, Bass/Tile).

out[b, d, h, w] = mean_c(L[b,c,h,w] * R[b,c,h,w-d]), d in [0,40), 0 where w<d.
Data-parallel over batch (B=8 -> 1 sample per core).

Pipeline: 8 load/compute chunks (hc=8), extraction per PAIR (hp=16).
1. Loads: lt/rt chunk [128c, (ch2, hc8, w128)] f32, bufs=3, 8KB runs.
   sync queue carries all loads except chunk-0's rt (scalar); chunk-0 is
   split into h-halves so the first matmuls start after half the bytes.
   Emission: loads(0..2), then per chunk j: compute(j), loads(j+3),
   ext(pair) - loads never sit behind extraction DMAs in a queue.
2. Per h: 2 accumulating fp32 matmuls -> PSUM Gram G[w, w'] (4 h/bank,
   7 banks rotating + 1 keepalive bank).
3. PE p-state keepalive: the PE clock ramps down when idle (peak needs
   >3us continuous activity) and per-chunk compute at peak (3.4us) <
   per-chunk load (5.8us). Dummy matmuls into the spare bank hold peak.
4. Gram copy (x 1/C, cast bf16) PSUM -> pair tile g[w, (hl16, 128)],
   split per 4-h group across DVE/ACT: partitions [0:64) get only cols
   [0:89) so the one-time zeroed tails (rows w<64, cols>=89) survive -
   the w<d triangle readback lands exactly there. One pair tile per
   parity (2 halves of a persistent g_big; chunk 2p fills hl 0:8,
   2p+1 fills hl 8:16).
5. Dump per pair on the SWDGE (gpsimd) queue - the idle Pool engine
   owns dump+readback+out so their producer waits never head-of-line
   block the scalar sequencer (dma_start dispatch blocks the issuing
   engine's SEQ in order). Contiguous 4KB runs per partition.
6. Band readback (+1-pitch trick): offset(hl, w, dr) = w*2049 + hl*128
   + dr lands on stored G[w, hl, w+dr-39]; w<d reads the zeroed tails.
   dst o_pre[hl(16 part), (w, dr)] bf16, 80B runs. The (hl=0, w=0,
   dr<39) corner reads the uninitialized scratch head - those outputs
   are identically 0 (w=0<d) and are re-zeroed after the reshuffle.
   (A tiny 78B DMA pre-zeroing the scratch head does NOT land reliably
   on this hardware - do not reintroduce it.)
7. Reshuffle+cast to o_t[hl, (d, w)] f32 from the reversed-dr view,
   entirely on DVE (keeps ACT free). Out DMA per pair on SWDGE,
   [hl, (d, w)] -> out[d, h, w], 512B runs. The last pair skips SWDGE
   (no descriptor-gen latency on the serial tail): scalar dump+rb,
   d-split reshuffle with outs interleaved on scalar.

Environment facts (measured this session):
- DMA is one shared device ~360 GB/s aggregate (16 engines x 22.5
  B/ns); queue count does NOT add bandwidth; runs <512B pay 2x; 7ns
  per-descriptor floor. SWDGE adds 994ns + 0.34ns/desc generation on
  the Pool engine but transfers at full speed.
- Compute-engine APs need 32-aligned partition bases (0/32/64/96).
- Wall timings through the axon terminal are extremely bursty
  (240us-1.1ms for identical kernels); only min-over-many-runs in a
  quiet window is meaningful. TimelineSim (sim.py) tracks the device
  time well: baseline 108.6us sim <-> 240.8us measured quiet-window
  (fixed dispatch overhead ~130-160us), this kernel sims at 76.5us.
"""

import numpy as np

C, H, W, D = 256, 64, 128, 40
PAD = 39                   # scratch head slack covering w+dr-39 < 0
N_CORES = 8
NCHUNK = 8                 # load/compute chunks
HC = H // NCHUNK           # 8 h per chunk
HP = 2 * HC                # 16 h per extraction pair
CGP = HP * W               # 2048 scratch row elems (write pitch)
RP = CGP + 1               # 2049 readback pitch
SSIZE = W * RP + 64        # scratch elems (covers the readback view)

_cache = {}


def _build(nchunk=NCHUNK, swdge=True, split_copies=True, keepalive=True,
           c0split=True):
    import concourse.bass as bass
    import concourse.bacc as bacc
    import concourse.mybir as mybir
    from concourse.tile import TileContext

    f32 = mybir.dt.float32
    bf16 = mybir.dt.bfloat16
    hc = H // nchunk
    hp = 2 * hc
    cgp = hp * W
    npair = nchunk // 2
    nc = bacc.Bacc("TRN2", target_bir_lowering=False, debug=False,
                   num_devices=N_CORES)
    l_in = nc.dram_tensor("l", [C, H, W], f32, kind="ExternalInput")
    r_in = nc.dram_tensor("r", [C, H, W], f32, kind="ExternalInput")
    out = nc.dram_tensor("out", [D, H, W], f32, kind="ExternalOutput")

    lv = l_in.ap().rearrange("(ch p) h w -> p ch h w", ch=2)
    rv = r_in.ap().rearrange("(ch p) h w -> p ch h w", ch=2)
    o4 = out.ap().rearrange("d (p hl) w -> p hl d w", hl=hp)

    with TileContext(nc) as tc:
        with (
            tc.tile_pool(name="inp", bufs=3) as inp,
            tc.tile_pool(name="gp", bufs=1) as gp,
            tc.tile_pool(name="ext", bufs=3) as ext,
            tc.tile_pool(name="ps", bufs=1, space="PSUM") as psp,
            tc.tile_pool(name="dram", bufs=1, space="DRAM") as dp,
        ):
            lts, rts = {}, {}

            def emit_loads(j):
                h0 = j * hc
                lt = inp.tile([128, 2 * hc * W], f32, tag="lt")
                rt = inp.tile([128, 2 * hc * W], f32, tag="rt")
                lt4 = lt[:, :].rearrange("p (ch h w) -> p ch h w",
                                         ch=2, h=hc)
                rt4 = rt[:, :].rearrange("p (ch h w) -> p ch h w",
                                         ch=2, h=hc)
                if j == 0 and c0split:
                    # split chunk-0 into h-halves so the first matmuls
                    # start after half the bytes
                    h2 = hc // 2
                    nc.sync.dma_start(lt4[:, :, 0:h2], lv[:, :, 0:h2, :])
                    nc.scalar.dma_start(rt4[:, :, 0:h2], rv[:, :, 0:h2, :])
                    nc.sync.dma_start(lt4[:, :, h2:], lv[:, :, h2:hc, :])
                    nc.scalar.dma_start(rt4[:, :, h2:], rv[:, :, h2:hc, :])
                elif j == 0:
                    nc.sync.dma_start(lt4, lv[:, :, h0:h0 + hc, :])
                    nc.scalar.dma_start(rt4, rv[:, :, h0:h0 + hc, :])
                else:
                    nc.sync.dma_start(lt4, lv[:, :, h0:h0 + hc, :])
                    nc.sync.dma_start(rt4, rv[:, :, h0:h0 + hc, :])
                lts[j], rts[j] = lt4, rt4

            def emit_compute(j, g_big):
                lt4, rt4 = lts[j], rts[j]
                phalf = (j // 2) % 2
                g_pair = g_big[:, phalf * cgp:(phalf + 1) * cgp]
                gp3 = g_pair.rearrange("w (h k) -> w h k", k=W)
                hbase = (j % 2) * hc
                for g in range(hc // 4):
                    gi = j * (hc // 4) + g
                    gm = psp.tile([128, 512], f32, tag=f"g{gi % 7}")
                    for k in range(4):
                        hb = g * 4 + k
                        for ch in range(2):
                            nc.tensor.matmul(
                                gm[:, 128 * k:128 * (k + 1)],
                                lt4[:, ch, hb, :], rt4[:, ch, hb, :],
                                start=(ch == 0), stop=(ch == 1),
                            )
                    # split copy keeps the one-time zeroed tails (rows w<64,
                    # cols>=89) intact: the w<d triangle readback lands there
                    dst = gp3[:, hbase + 4 * g:hbase + 4 * g + 4, :]
                    src = gm[:, :].rearrange("p (h w) -> p h w", h=4)
                    if not split_copies:
                        if (j + g) % 2 == 0:
                            nc.vector.tensor_scalar_mul(dst, src, 1.0 / C)
                        else:
                            nc.scalar.activation(
                                dst, src,
                                mybir.ActivationFunctionType.Copy,
                                scale=1.0 / C)
                    elif (j + g) % 2 == 0:
                        nc.vector.tensor_scalar_mul(
                            dst[0:64, :, 0:89], src[0:64, :, 0:89], 1.0 / C)
                        nc.scalar.activation(
                            dst[64:128], src[64:128],
                            mybir.ActivationFunctionType.Copy, scale=1.0 / C)
                    else:
                        nc.scalar.activation(
                            dst[0:64, :, 0:89], src[0:64, :, 0:89],
                            mybir.ActivationFunctionType.Copy, scale=1.0 / C)
                        nc.vector.tensor_scalar_mul(
                            dst[64:128], src[64:128], 1.0 / C)
                if not split_copies and j % 2 == 1:
                    # re-zero the readback tails clobbered by whole copies
                    nc.gpsimd.memset(gp3[0:PAD, :, 89:W], 0.0)
                return g_pair

            def emit_ext(p, g_pair, sc, last=False):
                # mid-stream pairs ride the SWDGE queue (idle Pool engine,
                # no scalar-sequencer head-of-line blocking); the last pair
                # uses the now-free scalar HWDGE queue to skip the ~2.7us
                # SWDGE descriptor-generation latency on its serial tail.
                dq = nc.scalar if (last or not swdge) else nc.gpsimd
                wv = sc[PAD:PAD + 128 * cgp].rearrange("(w f) -> w f", w=128)
                dq.dma_start(wv, g_pair)

                # readback: offset = w*(cgp+1) + hl*W + dr
                rbv = sc[0:128 * (cgp + 1)].rearrange("(w r) -> w r", w=128)
                rb3 = rbv[:, 0:hp * W].rearrange(
                    "w (hl k) -> w hl k", k=W)[:, :, 0:D]
                o_pre = ext.tile([hp, W * D], bf16, tag="opre")
                op3 = o_pre[:, :].rearrange("p (w dr) -> p w dr", dr=D)
                dq.dma_start(op3, rb3.transpose([1, 0, 2]))

                # reshuffle+cast to [hl, d, w] entirely on DVE (keeps ACT
                # free for gram copies - no cross-chain head-of-line
                # blocking); w<d triangle already zero. Mid-stream outs ride
                # SWDGE so the whole chain is Pool-owned; the last pair's
                # reshuffle splits in d-halves with outs interleaved on the
                # now-free scalar queue.
                o_t = ext.tile([hp, D * W], f32, tag="ot")
                ov = o_t[:, :].rearrange("p (d w) -> p d w", d=D)
                srcv = op3[:, :, ::-1].transpose([0, 2, 1])  # [hl, d, w]
                ot3 = o_t[:, :].rearrange("p (d w) -> p d w", d=D)
                # the (hl=0, w=0, d>0) cells read the uninitialized 39-elem
                # scratch head (the "row -1 tail"); they are identically 0
                # in the output (w=0 < d), so overwrite them after the copy
                if last:
                    nc.vector.tensor_copy(ov[:, 0:20], srcv[:, 0:20])
                    nc.vector.memset(ov[0:1, 1:20, 0:1], 0.0)
                    nc.scalar.dma_start(o4[p][:, 0:20], ot3[:, 0:20])
                    nc.vector.tensor_copy(ov[:, 20:D], srcv[:, 20:D])
                    nc.vector.memset(ov[0:1, 20:D, 0:1], 0.0)
                    nc.scalar.dma_start(o4[p][:, 20:D], ot3[:, 20:D])
                else:
                    nc.vector.tensor_copy(ov[:, 0:27], srcv[:, 0:27])
                    nc.scalar.activation(
                        ov[:, 27:D], srcv[:, 27:D],
                        mybir.ActivationFunctionType.Copy)
                    nc.vector.memset(ov[0:1, 1:D, 0:1], 0.0)
                    nc.scalar.dma_start(o4[p], ot3)

            scs = []
            for p in range(npair):
                sc = dp.tile([SSIZE], bf16, tag=f"sc{p}", name=f"sc{p}")
                scs.append(sc)

            # PE p-state keepalive: the PE clock ramps down when the engine
            # idles (peak needs >3us continuous activity). Per-chunk compute
            # (3.4us at peak) < per-chunk load time (5.8us), so without
            # filler the PE idles every chunk and drops to 2-3.7x slower
            # p-states. Tiny dummy matmuls into a sacrificial PSUM bank
            # keep the array busy during load waits.
            win = gp.tile([128, 128], f32, tag="win")
            nc.vector.memset(win[:, :], 0.0)
            warm = psp.tile([128, 512], f32, tag="warm")

            def emit_warm(n):
                for _ in range(n):
                    nc.tensor.matmul(warm[:, 0:64], win[:, :],
                                     win[:, 0:64], start=True, stop=True)

            # two pair tiles rotating via column halves; zero the tails
            # (rows w<64, cols>=89) once - rows w<64 never legitimately
            # read their own cols>=89, and the split gram copies never
            # overwrite them, so the w<d triangle readback reads zeros.
            g_big = gp.tile([128, 2 * cgp], bf16, tag="gbig")
            gz = g_big[:, :].rearrange("w (b h k) -> w b h k", b=2, k=W)
            nc.vector.memset(gz[0:64, :, :, 89:W], 0.0)

            emit_loads(0)
            emit_loads(1)
            emit_loads(2)
            if keepalive:
                emit_warm(55)
            for j in range(nchunk):
                g_pair = emit_compute(j, g_big)
                if j + 3 < nchunk:
                    emit_loads(j + 3)
                if j % 2 == 1:
                    emit_ext(j // 2, g_pair, scs[j // 2],
                             last=(j == nchunk - 1))
                if keepalive and j < nchunk - 1:
                    emit_warm(26)

    nc.compile()
    return nc


def _get_program():
    if "nc" not in _cache:
        _cache["nc"] = _build()
    return _cache["nc"]


def kernel(conv3a_l: np.ndarray, conv3a_r: np.ndarray) -> np.ndarray:
    from concourse import bass_utils

    nc = _get_program()
    conv3a_l = np.ascontiguousarray(conv3a_l, dtype=np.float32)
    conv3a_r = np.ascontiguousarray(conv3a_r, dtype=np.float32)
    in_maps = [
        {"l": conv3a_l[b], "r": conv3a_r[b]} for b in range(N_CORES)
    ]
    res = bass_utils.run_bass_kernel_spmd(nc, in_maps,
                                          core_ids=list(range(N_CORES)))
    return np.stack([res.results[b]["out"] for b in range(N_CORES)], axis=0)
